# revision 24
# baseline (speedup 1.0000x reference)
"""ChebyNet (K=3, 2 layers) on 8 Trainium2 NeuronCores.

Strategy:
  - Algebra: (A x) W == A (x W)  -> push features through weights first, so all
    4 SpMVs run at width 64. With A = -D^-1/2 Ahat D^-1/2, pre-scaling rows by
    dinv turns every SpMV into an unweighted gather+segment-sum.
  - Sharding: nodes globally sorted by degree, dealt round-robin to 8 cores
    (rank r -> core r%8, slot r//8) so every core sees an identical degree
    profile and one static NEFF schedule fits all cores. The 4 gather sources
    (scaled z / mid per layer) are exchanged with AllGather through shared
    DRAM as bf16 tables.
  - SpMV: ELL rounds per 128-node tile, PAIR-mode gather: table viewed as
    [V/2, 128] bf16 (two nodes per 256B row), idx = src>>1 fits a single int16
    window (25088 < 32768) -> no A/B window split, ~2.7% ELL padding only.
    Parity selection via a precomputed {0,1} bf16 mask multiply (DVE) with a
    stride-0 broadcast AP, then PE accumulates 8 sub-rounds per matmul into
    PSUM; gathers split across 4 SWDGE queues.
  - Padding gathers point at a zero pair (core0 pad slots 6250/6251).
"""
import os
import numpy as np
import ml_dtypes

BF16 = ml_dtypes.bfloat16
N, E, FIN, H, C = 50000, 800000, 100, 64, 40
NC = 8
P = 128
PC = 6272           # padded nodes per core (49*128; 6250 real)
T = PC // P         # 49 tiles
V = NC * PC         # 50176 table rows
VP = V // 2         # 25088 pair rows (int16-addressable)
PAD_PAIR = 3125     # pair (6250,6251): core0 zero pad rows
RB = 96             # rounds budget per gather batch


def _preprocess(x, edge_index):
    row = np.asarray(edge_index[0], dtype=np.int64)
    col = np.asarray(edge_index[1], dtype=np.int64)
    deg = np.bincount(row, minlength=N).astype(np.int64)
    order = np.argsort(-deg, kind="stable")        # rank -> node
    rank = np.empty(N, np.int64); rank[order] = np.arange(N)
    corer, posr = rank % NC, rank // NC
    znew = corer * PC + posr                       # node -> table row

    ecore = corer[row]; epos = posr[row]

    cnt = np.zeros((NC, PC), np.int64)
    np.add.at(cnt, (ecore, epos), 1)

    # shared static schedule: per-tile round counts = max over cores
    da = [int(cnt[:, t * P:(t + 1) * P].max()) for t in range(T)]

    # group tiles into gather batches with a round budget (SBUF staging cap)
    batches = []
    cur = []
    budget = 0
    for t in range(T):
        r = da[t]
        if cur and budget + r > RB:
            batches.append(cur); cur = []; budget = 0
        cur.append(t); budget += r
    if cur:
        batches.append(cur)
    # keep the final batch short: the AllGather (and thus the next spmv) waits
    # on the last epilogue, so a small tail batch shortens the exposed chain
    if len(batches[-1]) > 3:
        batches.append(batches[-1][-2:])
        batches[-2] = batches[-2][:-2]

    ag = None
    ecz = znew[col]
    pad_pair = PAD_PAIR

    RMAX = int(cnt.max())
    ell = np.full((NC, PC, RMAX), pad_pair, np.int32)
    msk = np.zeros((NC, PC, RMAX, 2), np.float32)
    for c in range(NC):
        m = ecore == c
        ep, ez = epos[m], ecz[m]
        o = np.argsort(ep, kind="stable")
        ep, ez = ep[o], ez[o]
        starts = np.searchsorted(ep, np.arange(PC))
        r = np.arange(len(ep)) - starts[ep]
        ell[c, ep, r] = ez >> 1
        msk[c, ep, r, (ez & 1)] = 1.0

    # index buffer: idx k of a gather goes to [k%16, off + k//16]; replicated
    # to 128 partitions. k = r*128 + p -> stage[p, r].
    def wrap(seq):
        return np.tile(seq.reshape(-1, 16).T.astype(np.int16), (8, 1))

    ICOLS = 8 * sum(da)
    MC = sum(da)
    idxbuf = np.zeros((NC, 128, ICOLS), np.int16)
    mskbuf = np.zeros((NC, 128, 2 * MC), np.float32)
    offs = []      # per batch: (idx col, mask round offset)
    off = 0
    moff = 0
    for bt in batches:
        offs.append((off, moff))
        for c in range(NC):
            o = off
            mo = moff
            for t in bt:
                seq = ell[c, t * P:(t + 1) * P, :da[t]].T.ravel()
                idxbuf[c, :, o:o + 8 * da[t]] = wrap(seq)
                mskbuf[c, :, 2 * mo:2 * (mo + da[t])] = (
                    msk[c, t * P:(t + 1) * P, :da[t], :].reshape(P, 2 * da[t]))
                o += 8 * da[t]
                mo += da[t]
        off += 8 * sum(da[t] for t in bt)
        moff += sum(da[t] for t in bt)
    offs = (batches, offs)

    deg_pt = np.zeros((NC, P, T), np.float32)
    xts = np.zeros((NC, FIN + 1, PC), np.float32)
    for c in range(NC):
        nq = min(PC, (N - c + NC - 1) // NC)  # real nodes on this core (6250)
        nodes = order[np.arange(nq) * NC + c]
        q = np.arange(nq)
        deg_pt[c, q % P, q // P] = deg[nodes]
        xts[c, :FIN, :nq] = x[nodes].T
        xts[c, FIN, :nq] = 1.0
    return order, da, offs, ag, idxbuf, mskbuf.astype(BF16), deg_pt, xts


def _build(da, offs, ag):
    import concourse.bacc as bacc
    import concourse.mybir as mybir
    import concourse.tile as tile
    from concourse.masks import make_identity
    f32 = mybir.dt.float32
    bf16 = mybir.dt.bfloat16

    nc = bacc.Bacc("TRN2", target_bir_lowering=False, debug=False, num_devices=NC,
                   num_swdge_queues=4)
    IC = 8 * sum(da)
    MC = sum(da)
    xt_d = nc.dram_tensor("xt", [FIN + 1, PC], f32, kind="ExternalInput")
    w1_d = nc.dram_tensor("w1p", [FIN + 1, 3 * H], f32, kind="ExternalInput")
    w2_d = nc.dram_tensor("w2p", [H + 1, 3 * C], f32, kind="ExternalInput")
    idx_d = nc.dram_tensor("idx", [128, IC], mybir.dt.int16, kind="ExternalInput")
    msk_d = nc.dram_tensor("msk", [128, 2 * MC], bf16, kind="ExternalInput")
    deg_d = nc.dram_tensor("deg", [P, T], f32, kind="ExternalInput")
    out_d = nc.dram_tensor("out", [PC, C], f32, kind="ExternalOutput")

    with tile.TileContext(nc) as tc:
        with (
            tc.tile_pool(name="cst", bufs=1) as cst,
            tc.tile_pool(name="sb", bufs=3) as sb,
            tc.tile_pool(name="stg", bufs=3) as stg,
            tc.tile_pool(name="ps", bufs=2, space="PSUM") as ps,
            tc.tile_pool(name="acc", bufs=3, space="PSUM") as accp,
            tc.tile_pool(name="dram", bufs=1, space="DRAM") as dram,
        ):
            ident = cst.tile([P, P], f32)
            make_identity(nc, ident[:])
            identb = cst.tile([P, P], bf16)
            nc.vector.tensor_copy(out=identb[:], in_=ident[:])
            w1 = cst.tile([FIN + 1, 3 * H], f32)
            nc.sync.dma_start(out=w1[:], in_=w1_d[:])
            w2 = cst.tile([H + 1, 3 * C], f32)
            nc.sync.dma_start(out=w2[:], in_=w2_d[:])
            idx = cst.tile([128, IC], mybir.dt.int16)
            nc.sync.dma_start(out=idx[:], in_=idx_d[:])
            mskc = cst.tile([128, 2 * MC], bf16)
            nc.sync.dma_start(out=mskc[:], in_=msk_d[:])
            deg = cst.tile([P, T], f32)
            nc.sync.dma_start(out=deg[:], in_=deg_d[:])
            M = mybir.AluOpType.mult
            # dinv = (deg>0) / sqrt(max(deg,1))
            dinv = cst.tile([P, T], f32)
            dinv2 = cst.tile([P, T], f32)
            tmpd = cst.tile([P, T], f32)
            nc.vector.tensor_scalar(out=tmpd[:], in0=deg[:], scalar1=1.0, scalar2=None,
                                    op0=mybir.AluOpType.max)
            nc.vector.reciprocal(out=tmpd[:], in_=tmpd[:])
            nc.scalar.sqrt(out=tmpd[:], in_=tmpd[:])
            mk = cst.tile([P, T], f32)
            nc.vector.tensor_scalar(out=mk[:], in0=deg[:], scalar1=0.5, scalar2=None,
                                    op0=mybir.AluOpType.is_ge)
            nc.vector.tensor_mul(out=dinv[:], in0=tmpd[:], in1=mk[:])
            nc.vector.tensor_mul(out=dinv2[:], in0=dinv[:], in1=dinv[:])
            negd = cst.tile([P, T], f32)
            negd2 = cst.tile([P, T], f32)
            nc.vector.tensor_scalar(out=negd[:], in0=dinv[:], scalar1=-1.0,
                                    scalar2=None, op0=M)
            nc.vector.tensor_scalar(out=negd2[:], in0=dinv2[:], scalar1=-1.0,
                                    scalar2=None, op0=M)

            vt0buf = cst.tile([P, T, 2 * H], f32)   # [v | t0] per tile
            vt2buf = cst.tile([P, T, 2 * C], f32)   # [v2 | t02] per tile

            agin = [dram.tile([PC, H], bf16, tag=f"agin{i}", name=f"agin{i}")
                    for i in range(4)]
            tabs = [dram.tile([V, H], bf16, addr_space="Shared", tag=f"tab{i}",
                              name=f"tab{i}") for i in range(4)]

            def allgather(i):
                nc.gpsimd.collective_compute(
                    "AllGather", mybir.AluOpType.bypass,
                    replica_groups=[list(range(NC))],
                    ins=[agin[i][:].opt()], outs=[tabs[i][:].opt()])

            batches, boffs = offs

            def spmv(i, epilogue, agnext=None):
                """run spmv i over all tiles; epilogue(t, acc_psum) per tile;
                AllGather for table `agnext` fires after the last batch."""
                tabv = tabs[i][:].rearrange("(v two) c -> v (two c)", two=2)
                for g, bt in enumerate(batches):
                    rs = sum(da[t] for t in bt)
                    col, mo = boffs[g]
                    st = stg.tile([P, rs, 2 * H], bf16, tag="stage", name="st")
                    # split the batch's rounds across the 4 SWDGE queues
                    qsp = [rs * q // 4 for q in range(5)]
                    for q in range(4):
                        r0, r1 = qsp[q], qsp[q + 1]
                        if r1 == r0:
                            continue
                        nc.gpsimd.dma_gather(
                            out_ap=st[:, r0:r1, :], in_ap=tabv,
                            idxs_ap=idx[:, col + 8 * r0:col + 8 * r1],
                            num_idxs=(r1 - r0) * P, num_idxs_reg=(r1 - r0) * P,
                            elem_size=2 * H, single_packet=False, queue_num=q)
                    # parity select: st[p, r, s, c] *= msk[p, r, s]
                    stv4 = st[:].rearrange("p r (s c) -> p r s c", s=2)
                    mv4 = (mskc[:, 2 * mo:2 * (mo + rs)]
                           .rearrange("p (r s) -> p r s", s=2)
                           .unsqueeze(-1).broadcast_to([P, rs, 2, H]))
                    nc.vector.tensor_mul(out=stv4, in0=stv4, in1=mv4)
                    stv = st[:].rearrange("p r (s c) -> p (r s) c", s=2)
                    ao = 0
                    for t in bt:
                        sr = 2 * da[t]       # sub-rounds for this tile
                        p512 = accp.tile([P, 8, H], f32, tag="acc", space="PSUM",
                                         name="p512")
                        chunks = [(sc, min(8, sr - sc)) for sc in range(0, sr, 8)]
                        cov = chunks[0][1]
                        for k, (sc, r) in enumerate(chunks):
                            nc.tensor.matmul(out=p512[:, 0:r, :], lhsT=identb[:],
                                             rhs=stv[:, ao + sc:ao + sc + r, :],
                                             start=(k == 0),
                                             stop=(k == len(chunks) - 1))
                        acc = sb.tile([P, H], f32, tag="accs", name="accs")
                        nc.vector.tensor_reduce(
                            out=acc[:],
                            in_=p512[:, 0:cov, :].rearrange("p a b -> p b a"),
                            axis=mybir.AxisListType.X, op=mybir.AluOpType.add)
                        ao += sr
                        epilogue(t, acc)
                if agnext is not None:
                    allgather(agnext)

            dc = lambda t: dinv[:, t:t + 1]
            ndc = lambda t: negd[:, t:t + 1]
            nd2c = lambda t: negd2[:, t:t + 1]

            # ---- layer 1 matmuls: [2*W1[2] | W1[1] | W1[0]-W1[2] (+b1)] ----
            xall = cst.tile([FIN + 1, PC], f32)
            nc.sync.dma_start(out=xall[:], in_=xt_d[:])
            for t in range(T):
                pm = ps.tile([P, 3 * H], f32, tag="mm", space="PSUM")
                nc.tensor.matmul(out=pm[:], lhsT=xall[:, t * P:(t + 1) * P],
                                 rhs=w1[:], start=True, stop=True)
                zt = sb.tile([P, H], bf16, tag="zt")
                nc.scalar.mul(out=zt[:], in_=pm[:, 0:H], mul=dc(t))
                nc.sync.dma_start(out=agin[0][t * P:(t + 1) * P, :], in_=zt[:])
                nc.vector.tensor_copy(out=vt0buf[:, t, :], in_=pm[:, H:3 * H])
            allgather(0)

            # ---- spmv1 -> mhat -> AG2 ----
            def epi1(t, acc):
                mh = sb.tile([P, H], f32, tag="mh", name="mh")
                nc.scalar.mul(out=mh[:], in_=acc[:], mul=nd2c(t))
                tv = sb.tile([P, H], f32, tag="tv", name="tv")
                nc.scalar.mul(out=tv[:], in_=vt0buf[:, t, 0:H], mul=dc(t))
                mhb = sb.tile([P, H], bf16, tag="mhb", name="mhb")
                nc.vector.tensor_add(out=mhb[:], in0=mh[:], in1=tv[:])
                nc.sync.dma_start(out=agin[1][t * P:(t + 1) * P, :], in_=mhb[:])
            spmv(0, epi1, agnext=1)

            # ---- spmv2 -> h = relu(t0 - dinv*s) -> layer 2 matmuls (fused) ----
            def epi2(t, acc):
                hb = sb.tile([P, H], f32, tag="hb", name="hb")
                nc.scalar.mul(out=hb[:], in_=acc[:], mul=ndc(t))
                nc.vector.tensor_add(out=hb[:], in0=hb[:], in1=vt0buf[:, t, H:2 * H])
                nc.vector.tensor_scalar(out=hb[:], in0=hb[:],
                                        scalar1=0.0, scalar2=None,
                                        op0=mybir.AluOpType.max)
                pt = ps.tile([H, P], f32, tag="tr", space="PSUM")
                nc.tensor.transpose(out=pt[:], in_=hb[:], identity=ident[:])
                ht = sb.tile([H + 1, P], f32, tag="ht")
                nc.vector.tensor_copy(out=ht[0:H, :], in_=pt[:])
                nc.vector.memset(ht[H:H + 1, :], 1.0)
                pm = ps.tile([P, 3 * C], f32, tag="mm", space="PSUM")
                nc.tensor.matmul(out=pm[:], lhsT=ht[:], rhs=w2[:], start=True, stop=True)
                z2 = sb.tile([P, H], bf16, tag="z2")
                nc.vector.memset(z2[:, C:H], 0.0)
                nc.scalar.mul(out=z2[:, 0:C], in_=pm[:, 0:C], mul=dc(t))
                nc.sync.dma_start(out=agin[2][t * P:(t + 1) * P, :], in_=z2[:])
                nc.vector.tensor_copy(out=vt2buf[:, t, :], in_=pm[:, C:3 * C])
            spmv(1, epi2, agnext=2)

            # ---- spmv3 -> mhat2 -> AG4 ----
            def epi3(t, acc):
                m2 = sb.tile([P, H], bf16, tag="m2", name="m2")
                nc.vector.memset(m2[:, C:H], 0.0)
                mt = sb.tile([P, C], f32, tag="mt", name="mt")
                nc.scalar.mul(out=mt[:], in_=acc[:, 0:C], mul=nd2c(t))
                tv = sb.tile([P, C], f32, tag="tv2", name="tv")
                nc.scalar.mul(out=tv[:], in_=vt2buf[:, t, 0:C], mul=dc(t))
                nc.vector.tensor_add(out=m2[:, 0:C], in0=mt[:], in1=tv[:])
                nc.sync.dma_start(out=agin[3][t * P:(t + 1) * P, :], in_=m2[:])
            spmv(2, epi3, agnext=3)

            # ---- spmv4 -> logits -> log_softmax -> out ----
            def epi4(t, acc):
                lg = sb.tile([P, C], f32, tag="lg", name="lg")
                nc.scalar.mul(out=lg[:], in_=acc[:, 0:C], mul=ndc(t))
                nc.vector.tensor_add(out=lg[:], in0=lg[:], in1=vt2buf[:, t, C:2 * C])
                nmx = sb.tile([P, 1], f32, tag="nmx", name="nmx")
                nc.vector.tensor_reduce(out=nmx[:], in_=lg[:],
                                        axis=mybir.AxisListType.X,
                                        op=mybir.AluOpType.max, negate=True)
                e1 = sb.tile([P, C], f32, tag="e1", name="e1")
                nc.scalar.activation(out=e1[:], in_=lg[:],
                                     func=mybir.ActivationFunctionType.Identity,
                                     bias=nmx[:], scale=1.0)
                ex = sb.tile([P, C], f32, tag="ex", name="ex")
                sm = sb.tile([P, 1], f32, tag="sm", name="sm")
                nc.scalar.activation(out=ex[:], in_=e1[:],
                                     func=mybir.ActivationFunctionType.Exp,
                                     accum_out=sm[:])
                rs = sb.tile([P, 1], f32, tag="rs", name="rs")
                nc.vector.reciprocal(out=rs[:], in_=sm[:])
                nls = sb.tile([P, 1], f32, tag="nls", name="nls")
                nc.scalar.activation(out=nls[:], in_=rs[:],
                                     func=mybir.ActivationFunctionType.Ln)
                ot = sb.tile([P, C], f32, tag="ot", name="ot")
                nc.scalar.activation(out=ot[:], in_=e1[:],
                                     func=mybir.ActivationFunctionType.Identity,
                                     bias=nls[:], scale=1.0)
                nc.sync.dma_start(out=out_d[t * P:(t + 1) * P, :], in_=ot[:])
            spmv(3, epi4)
    nc.compile()
    return nc


def kernel(x, edge_index, W1, b1, W2, b2):
    x = np.asarray(x, np.float32)
    W1 = np.asarray(W1, np.float32); b1 = np.asarray(b1, np.float32)
    W2 = np.asarray(W2, np.float32); b2 = np.asarray(b2, np.float32)

    order, da, offs, ag, idxbuf, mskbuf, deg_pt, xts = _preprocess(x, edge_index)

    w1p = np.zeros((FIN + 1, 3 * H), np.float32)
    w1p[:FIN, 0:H] = 2.0 * W1[2]
    w1p[:FIN, H:2 * H] = W1[1]
    w1p[:FIN, 2 * H:3 * H] = W1[0] - W1[2]
    w1p[FIN, 2 * H:3 * H] = b1
    w2p = np.zeros((H + 1, 3 * C), np.float32)
    w2p[:H, 0:C] = 2.0 * W2[2]
    w2p[:H, C:2 * C] = W2[1]
    w2p[:H, 2 * C:3 * C] = W2[0] - W2[2]
    w2p[H, 2 * C:3 * C] = b2

    trace = bool(os.environ.get("CHEB_TRACE"))
    if trace:
        import sys, types
        try:
            from trn_agent_boot.trn_boot import _ntff_profile_via_ctypes
            m = types.ModuleType("antenv.axon_hooks")
            m.get_axon_ntff_profile_hook = (
                lambda: _ntff_profile_via_ctypes("/opt/axon/libaxon_pjrt.so"))
            sys.modules["antenv.axon_hooks"] = m
        except Exception:
            trace = False

    nc = _build(da, offs, ag)
    from concourse.bass_utils import run_bass_kernel_spmd
    ins = [{"xt": xts[c], "w1p": w1p, "w2p": w2p, "idx": idxbuf[c],
            "msk": mskbuf[c], "deg": deg_pt[c]} for c in range(NC)]
    res = run_bass_kernel_spmd(nc, ins, core_ids=list(range(NC)), trace=trace)
    if trace and res.exec_time_ns is not None:
        print(f"HW exec time: {res.exec_time_ns} ns")

    out = np.empty((N, C), np.float32)
    for c in range(NC):
        nq = min(PC, (N - c + NC - 1) // NC)
        nodes = order[np.arange(nq) * NC + c]
        out[nodes] = res.results[c]["out"][:nq]
    return out


# revision 25
# speedup vs baseline: 1.0719x; 1.0719x over previous
"""ChebyNet (K=3, 2 layers) on 8 Trainium2 NeuronCores.

Strategy:
  - Algebra: (A x) W == A (x W)  -> push features through weights first, so all
    4 SpMVs run at width 64. With A = -D^-1/2 Ahat D^-1/2, pre-scaling rows by
    dinv turns every SpMV into an unweighted gather+segment-sum.
  - Sharding: nodes globally sorted by degree, dealt round-robin to 8 cores
    (rank r -> core r%8, slot r//8) so every core sees an identical degree
    profile and one static NEFF schedule fits all cores. The 4 gather sources
    (scaled z / mid per layer) are exchanged with AllGather through shared
    DRAM as bf16 tables.
  - SpMV: ELL rounds per 128-node tile, PAIR-mode gather: table viewed as
    [V/2, 128] bf16 (two nodes per 256B row), idx = src>>1 fits a single int16
    window (25088 < 32768) -> no A/B window split, ~2.7% ELL padding only.
    Parity selection via a precomputed {0,1} bf16 mask multiply (DVE) with a
    stride-0 broadcast AP, then PE accumulates 8 sub-rounds per matmul into
    PSUM; gathers split across 4 SWDGE queues.
  - Padding gathers point at a zero pair (core0 pad slots 6250/6251).
"""
import os
import numpy as np
import ml_dtypes

BF16 = ml_dtypes.bfloat16
N, E, FIN, H, C = 50000, 800000, 100, 64, 40
NC = 8
P = 128
PC = 6272           # padded nodes per core (49*128; 6250 real)
T = PC // P         # 49 tiles
V = NC * PC         # 50176 table rows
VP = V // 2         # 25088 pair rows (int16-addressable)
PAD_PAIR = 3125     # pair (6250,6251): core0 zero pad rows
RB = 64             # rounds budget per gather batch


def _preprocess(x, edge_index):
    row = np.asarray(edge_index[0], dtype=np.int64)
    col = np.asarray(edge_index[1], dtype=np.int64)
    deg = np.bincount(row, minlength=N).astype(np.int64)
    order = np.argsort(-deg, kind="stable")        # rank -> node
    rank = np.empty(N, np.int64); rank[order] = np.arange(N)
    corer, posr = rank % NC, rank // NC
    znew = corer * PC + posr                       # node -> table row

    ecore = corer[row]; epos = posr[row]

    cnt = np.zeros((NC, PC), np.int64)
    np.add.at(cnt, (ecore, epos), 1)

    # shared static schedule: per-tile round counts = max over cores
    da = [int(cnt[:, t * P:(t + 1) * P].max()) for t in range(T)]

    # group tiles into gather batches with a round budget (SBUF staging cap)
    batches = []
    cur = []
    budget = 0
    for t in range(T):
        r = da[t]
        if cur and budget + r > RB:
            batches.append(cur); cur = []; budget = 0
        cur.append(t); budget += r
    if cur:
        batches.append(cur)

    ag = None
    ecz = znew[col]
    pad_pair = PAD_PAIR

    RMAX = int(cnt.max())
    ell = np.full((NC, PC, RMAX), pad_pair, np.int32)
    msk = np.zeros((NC, PC, RMAX, 2), np.float32)
    for c in range(NC):
        m = ecore == c
        ep, ez = epos[m], ecz[m]
        o = np.argsort(ep, kind="stable")
        ep, ez = ep[o], ez[o]
        starts = np.searchsorted(ep, np.arange(PC))
        r = np.arange(len(ep)) - starts[ep]
        ell[c, ep, r] = ez >> 1
        msk[c, ep, r, (ez & 1)] = 1.0

    # index buffer: idx k of a gather goes to [k%16, off + k//16]; replicated
    # to 128 partitions. k = r*128 + p -> stage[p, r].
    def wrap(seq):
        return np.tile(seq.reshape(-1, 16).T.astype(np.int16), (8, 1))

    ICOLS = 8 * sum(da)
    MC = sum(da)
    idxbuf = np.zeros((NC, 128, ICOLS), np.int16)
    mskbuf = np.zeros((NC, 128, 2 * MC), np.float32)
    offs = []      # per batch: (idx col, mask round offset)
    off = 0
    moff = 0
    for bt in batches:
        offs.append((off, moff))
        for c in range(NC):
            o = off
            mo = moff
            for t in bt:
                seq = ell[c, t * P:(t + 1) * P, :da[t]].T.ravel()
                idxbuf[c, :, o:o + 8 * da[t]] = wrap(seq)
                mskbuf[c, :, 2 * mo:2 * (mo + da[t])] = (
                    msk[c, t * P:(t + 1) * P, :da[t], :].reshape(P, 2 * da[t]))
                o += 8 * da[t]
                mo += da[t]
        off += 8 * sum(da[t] for t in bt)
        moff += sum(da[t] for t in bt)
    offs = (batches, offs)

    deg_pt = np.zeros((NC, P, T), np.float32)
    xts = np.zeros((NC, FIN + 1, PC), np.float32)
    for c in range(NC):
        nq = min(PC, (N - c + NC - 1) // NC)  # real nodes on this core (6250)
        nodes = order[np.arange(nq) * NC + c]
        q = np.arange(nq)
        deg_pt[c, q % P, q // P] = deg[nodes]
        xts[c, :FIN, :nq] = x[nodes].T
        xts[c, FIN, :nq] = 1.0
    return order, da, offs, ag, idxbuf, mskbuf.astype(BF16), deg_pt, xts


def _build(da, offs, ag):
    import concourse.bacc as bacc
    import concourse.mybir as mybir
    import concourse.tile as tile
    from concourse.masks import make_identity
    f32 = mybir.dt.float32
    bf16 = mybir.dt.bfloat16

    nc = bacc.Bacc("TRN2", target_bir_lowering=False, debug=False, num_devices=NC,
                   num_swdge_queues=4)
    IC = 8 * sum(da)
    MC = sum(da)
    xt_d = nc.dram_tensor("xt", [FIN + 1, PC], f32, kind="ExternalInput")
    w1_d = nc.dram_tensor("w1p", [FIN + 1, 3 * H], f32, kind="ExternalInput")
    w2_d = nc.dram_tensor("w2p", [H + 1, 3 * C], f32, kind="ExternalInput")
    idx_d = nc.dram_tensor("idx", [128, IC], mybir.dt.int16, kind="ExternalInput")
    msk_d = nc.dram_tensor("msk", [128, 2 * MC], bf16, kind="ExternalInput")
    deg_d = nc.dram_tensor("deg", [P, T], f32, kind="ExternalInput")
    out_d = nc.dram_tensor("out", [PC, C], f32, kind="ExternalOutput")

    with tile.TileContext(nc) as tc:
        with (
            tc.tile_pool(name="cst", bufs=1) as cst,
            tc.tile_pool(name="sb", bufs=3) as sb,
            tc.tile_pool(name="stg", bufs=3) as stg,
            tc.tile_pool(name="ps", bufs=2, space="PSUM") as ps,
            tc.tile_pool(name="acc", bufs=3, space="PSUM") as accp,
            tc.tile_pool(name="dram", bufs=1, space="DRAM") as dram,
        ):
            ident = cst.tile([P, P], f32)
            make_identity(nc, ident[:])
            identb = cst.tile([P, P], bf16)
            nc.vector.tensor_copy(out=identb[:], in_=ident[:])
            w1 = cst.tile([FIN + 1, 3 * H], f32)
            nc.sync.dma_start(out=w1[:], in_=w1_d[:])
            w2 = cst.tile([H + 1, 3 * C], f32)
            nc.sync.dma_start(out=w2[:], in_=w2_d[:])
            idx = cst.tile([128, IC], mybir.dt.int16)
            nc.sync.dma_start(out=idx[:], in_=idx_d[:])
            mskc = cst.tile([128, 2 * MC], bf16)
            nc.sync.dma_start(out=mskc[:], in_=msk_d[:])
            deg = cst.tile([P, T], f32)
            nc.sync.dma_start(out=deg[:], in_=deg_d[:])
            M = mybir.AluOpType.mult
            # dinv = (deg>0) / sqrt(max(deg,1))
            dinv = cst.tile([P, T], f32)
            dinv2 = cst.tile([P, T], f32)
            tmpd = cst.tile([P, T], f32)
            nc.vector.tensor_scalar(out=tmpd[:], in0=deg[:], scalar1=1.0, scalar2=None,
                                    op0=mybir.AluOpType.max)
            nc.vector.reciprocal(out=tmpd[:], in_=tmpd[:])
            nc.scalar.sqrt(out=tmpd[:], in_=tmpd[:])
            mk = cst.tile([P, T], f32)
            nc.vector.tensor_scalar(out=mk[:], in0=deg[:], scalar1=0.5, scalar2=None,
                                    op0=mybir.AluOpType.is_ge)
            nc.vector.tensor_mul(out=dinv[:], in0=tmpd[:], in1=mk[:])
            nc.vector.tensor_mul(out=dinv2[:], in0=dinv[:], in1=dinv[:])
            negd = cst.tile([P, T], f32)
            negd2 = cst.tile([P, T], f32)
            nc.vector.tensor_scalar(out=negd[:], in0=dinv[:], scalar1=-1.0,
                                    scalar2=None, op0=M)
            nc.vector.tensor_scalar(out=negd2[:], in0=dinv2[:], scalar1=-1.0,
                                    scalar2=None, op0=M)

            vt0buf = cst.tile([P, T, 2 * H], f32)   # [v | t0] per tile
            vt2buf = cst.tile([P, T, 2 * C], f32)   # [v2 | t02] per tile

            agin = [dram.tile([PC, H], bf16, tag=f"agin{i}", name=f"agin{i}")
                    for i in range(4)]
            tabs = [dram.tile([V, H], bf16, addr_space="Shared", tag=f"tab{i}",
                              name=f"tab{i}") for i in range(4)]

            def allgather(i):
                nc.gpsimd.collective_compute(
                    "AllGather", mybir.AluOpType.bypass,
                    replica_groups=[list(range(NC))],
                    ins=[agin[i][:].opt()], outs=[tabs[i][:].opt()])

            batches, boffs = offs

            def spmv(i, epilogue, agnext=None):
                """run spmv i over all tiles; epilogue(t, acc_psum) per tile;
                AllGather for table `agnext` fires after the last batch."""
                tabv = tabs[i][:].rearrange("(v two) c -> v (two c)", two=2)
                for g, bt in enumerate(batches):
                    rs = sum(da[t] for t in bt)
                    col, mo = boffs[g]
                    st = stg.tile([P, rs, 2 * H], bf16, tag="stage", name="st")
                    # split the batch's rounds across the 4 SWDGE queues
                    qsp = [rs * q // 4 for q in range(5)]
                    for q in range(4):
                        r0, r1 = qsp[q], qsp[q + 1]
                        if r1 == r0:
                            continue
                        nc.gpsimd.dma_gather(
                            out_ap=st[:, r0:r1, :], in_ap=tabv,
                            idxs_ap=idx[:, col + 8 * r0:col + 8 * r1],
                            num_idxs=(r1 - r0) * P, num_idxs_reg=(r1 - r0) * P,
                            elem_size=2 * H, single_packet=False, queue_num=q)
                    # parity select: st[p, r, s, c] *= msk[p, r, s]
                    stv4 = st[:].rearrange("p r (s c) -> p r s c", s=2)
                    mv4 = (mskc[:, 2 * mo:2 * (mo + rs)]
                           .rearrange("p (r s) -> p r s", s=2)
                           .unsqueeze(-1).broadcast_to([P, rs, 2, H]))
                    nc.vector.tensor_mul(out=stv4, in0=stv4, in1=mv4)
                    stv = st[:].rearrange("p r (s c) -> p (r s) c", s=2)
                    ao = 0
                    for t in bt:
                        sr = 2 * da[t]       # sub-rounds for this tile
                        p512 = accp.tile([P, 8, H], f32, tag="acc", space="PSUM",
                                         name="p512")
                        chunks = [(sc, min(8, sr - sc)) for sc in range(0, sr, 8)]
                        cov = chunks[0][1]
                        for k, (sc, r) in enumerate(chunks):
                            nc.tensor.matmul(out=p512[:, 0:r, :], lhsT=identb[:],
                                             rhs=stv[:, ao + sc:ao + sc + r, :],
                                             start=(k == 0),
                                             stop=(k == len(chunks) - 1))
                        acc = sb.tile([P, H], f32, tag="accs", name="accs")
                        nc.vector.tensor_reduce(
                            out=acc[:],
                            in_=p512[:, 0:cov, :].rearrange("p a b -> p b a"),
                            axis=mybir.AxisListType.X, op=mybir.AluOpType.add)
                        ao += sr
                        epilogue(t, acc)
                if agnext is not None:
                    allgather(agnext)

            dc = lambda t: dinv[:, t:t + 1]
            ndc = lambda t: negd[:, t:t + 1]
            nd2c = lambda t: negd2[:, t:t + 1]

            # ---- layer 1 matmuls: [2*W1[2] | W1[1] | W1[0]-W1[2] (+b1)] ----
            for t in range(T):
                xt = sb.tile([FIN + 1, P], f32, tag="xt")
                nc.sync.dma_start(out=xt[:], in_=xt_d[:, t * P:(t + 1) * P])
                pm = ps.tile([P, 3 * H], f32, tag="mm", space="PSUM")
                nc.tensor.matmul(out=pm[:], lhsT=xt[:], rhs=w1[:], start=True, stop=True)
                zt = sb.tile([P, H], bf16, tag="zt")
                nc.scalar.mul(out=zt[:], in_=pm[:, 0:H], mul=dc(t))
                nc.sync.dma_start(out=agin[0][t * P:(t + 1) * P, :], in_=zt[:])
                nc.vector.tensor_copy(out=vt0buf[:, t, :], in_=pm[:, H:3 * H])
            allgather(0)

            # ---- spmv1 -> mhat -> AG2 ----
            def epi1(t, acc):
                mh = sb.tile([P, H], f32, tag="mh", name="mh")
                nc.scalar.mul(out=mh[:], in_=acc[:], mul=nd2c(t))
                tv = sb.tile([P, H], f32, tag="tv", name="tv")
                nc.scalar.mul(out=tv[:], in_=vt0buf[:, t, 0:H], mul=dc(t))
                mhb = sb.tile([P, H], bf16, tag="mhb", name="mhb")
                nc.vector.tensor_add(out=mhb[:], in0=mh[:], in1=tv[:])
                nc.sync.dma_start(out=agin[1][t * P:(t + 1) * P, :], in_=mhb[:])
            spmv(0, epi1, agnext=1)

            # ---- spmv2 -> h = relu(t0 - dinv*s) -> layer 2 matmuls (fused) ----
            def epi2(t, acc):
                hb = sb.tile([P, H], f32, tag="hb", name="hb")
                nc.scalar.mul(out=hb[:], in_=acc[:], mul=ndc(t))
                nc.vector.tensor_add(out=hb[:], in0=hb[:], in1=vt0buf[:, t, H:2 * H])
                nc.vector.tensor_scalar(out=hb[:], in0=hb[:],
                                        scalar1=0.0, scalar2=None,
                                        op0=mybir.AluOpType.max)
                pt = ps.tile([H, P], f32, tag="tr", space="PSUM")
                nc.tensor.transpose(out=pt[:], in_=hb[:], identity=ident[:])
                ht = sb.tile([H + 1, P], f32, tag="ht")
                nc.vector.tensor_copy(out=ht[0:H, :], in_=pt[:])
                nc.vector.memset(ht[H:H + 1, :], 1.0)
                pm = ps.tile([P, 3 * C], f32, tag="mm", space="PSUM")
                nc.tensor.matmul(out=pm[:], lhsT=ht[:], rhs=w2[:], start=True, stop=True)
                z2 = sb.tile([P, H], bf16, tag="z2")
                nc.vector.memset(z2[:, C:H], 0.0)
                nc.scalar.mul(out=z2[:, 0:C], in_=pm[:, 0:C], mul=dc(t))
                nc.sync.dma_start(out=agin[2][t * P:(t + 1) * P, :], in_=z2[:])
                nc.vector.tensor_copy(out=vt2buf[:, t, :], in_=pm[:, C:3 * C])
            spmv(1, epi2, agnext=2)

            # ---- spmv3 -> mhat2 -> AG4 ----
            def epi3(t, acc):
                m2 = sb.tile([P, H], bf16, tag="m2", name="m2")
                nc.vector.memset(m2[:, C:H], 0.0)
                mt = sb.tile([P, C], f32, tag="mt", name="mt")
                nc.scalar.mul(out=mt[:], in_=acc[:, 0:C], mul=nd2c(t))
                tv = sb.tile([P, C], f32, tag="tv2", name="tv")
                nc.scalar.mul(out=tv[:], in_=vt2buf[:, t, 0:C], mul=dc(t))
                nc.vector.tensor_add(out=m2[:, 0:C], in0=mt[:], in1=tv[:])
                nc.sync.dma_start(out=agin[3][t * P:(t + 1) * P, :], in_=m2[:])
            spmv(2, epi3, agnext=3)

            # ---- spmv4 -> logits -> log_softmax -> out ----
            def epi4(t, acc):
                lg = sb.tile([P, C], f32, tag="lg", name="lg")
                nc.scalar.mul(out=lg[:], in_=acc[:, 0:C], mul=ndc(t))
                nc.vector.tensor_add(out=lg[:], in0=lg[:], in1=vt2buf[:, t, C:2 * C])
                nmx = sb.tile([P, 1], f32, tag="nmx", name="nmx")
                nc.vector.tensor_reduce(out=nmx[:], in_=lg[:],
                                        axis=mybir.AxisListType.X,
                                        op=mybir.AluOpType.max, negate=True)
                e1 = sb.tile([P, C], f32, tag="e1", name="e1")
                nc.scalar.activation(out=e1[:], in_=lg[:],
                                     func=mybir.ActivationFunctionType.Identity,
                                     bias=nmx[:], scale=1.0)
                ex = sb.tile([P, C], f32, tag="ex", name="ex")
                sm = sb.tile([P, 1], f32, tag="sm", name="sm")
                nc.scalar.activation(out=ex[:], in_=e1[:],
                                     func=mybir.ActivationFunctionType.Exp,
                                     accum_out=sm[:])
                rs = sb.tile([P, 1], f32, tag="rs", name="rs")
                nc.vector.reciprocal(out=rs[:], in_=sm[:])
                nls = sb.tile([P, 1], f32, tag="nls", name="nls")
                nc.scalar.activation(out=nls[:], in_=rs[:],
                                     func=mybir.ActivationFunctionType.Ln)
                ot = sb.tile([P, C], f32, tag="ot", name="ot")
                nc.scalar.activation(out=ot[:], in_=e1[:],
                                     func=mybir.ActivationFunctionType.Identity,
                                     bias=nls[:], scale=1.0)
                nc.sync.dma_start(out=out_d[t * P:(t + 1) * P, :], in_=ot[:])
            spmv(3, epi4)
    nc.compile()
    return nc


def kernel(x, edge_index, W1, b1, W2, b2):
    x = np.asarray(x, np.float32)
    W1 = np.asarray(W1, np.float32); b1 = np.asarray(b1, np.float32)
    W2 = np.asarray(W2, np.float32); b2 = np.asarray(b2, np.float32)

    order, da, offs, ag, idxbuf, mskbuf, deg_pt, xts = _preprocess(x, edge_index)

    w1p = np.zeros((FIN + 1, 3 * H), np.float32)
    w1p[:FIN, 0:H] = 2.0 * W1[2]
    w1p[:FIN, H:2 * H] = W1[1]
    w1p[:FIN, 2 * H:3 * H] = W1[0] - W1[2]
    w1p[FIN, 2 * H:3 * H] = b1
    w2p = np.zeros((H + 1, 3 * C), np.float32)
    w2p[:H, 0:C] = 2.0 * W2[2]
    w2p[:H, C:2 * C] = W2[1]
    w2p[:H, 2 * C:3 * C] = W2[0] - W2[2]
    w2p[H, 2 * C:3 * C] = b2

    trace = bool(os.environ.get("CHEB_TRACE"))
    if trace:
        import sys, types
        try:
            from trn_agent_boot.trn_boot import _ntff_profile_via_ctypes
            m = types.ModuleType("antenv.axon_hooks")
            m.get_axon_ntff_profile_hook = (
                lambda: _ntff_profile_via_ctypes("/opt/axon/libaxon_pjrt.so"))
            sys.modules["antenv.axon_hooks"] = m
        except Exception:
            trace = False

    nc = _build(da, offs, ag)
    from concourse.bass_utils import run_bass_kernel_spmd
    ins = [{"xt": xts[c], "w1p": w1p, "w2p": w2p, "idx": idxbuf[c],
            "msk": mskbuf[c], "deg": deg_pt[c]} for c in range(NC)]
    res = run_bass_kernel_spmd(nc, ins, core_ids=list(range(NC)), trace=trace)
    if trace and res.exec_time_ns is not None:
        print(f"HW exec time: {res.exec_time_ns} ns")

    out = np.empty((N, C), np.float32)
    for c in range(NC):
        nq = min(PC, (N - c + NC - 1) // NC)
        nodes = order[np.arange(nq) * NC + c]
        out[nodes] = res.results[c]["out"][:nq]
    return out


# revision 26
# speedup vs baseline: 1.1242x; 1.0488x over previous
"""ChebyNet (K=3, 2 layers) on 8 Trainium2 NeuronCores.

Strategy:
  - Algebra: (A x) W == A (x W)  -> push features through weights first, so all
    4 SpMVs run at width 64. With A = -D^-1/2 Ahat D^-1/2, pre-scaling rows by
    dinv turns every SpMV into an unweighted gather+segment-sum.
  - Sharding: nodes globally sorted by degree, dealt round-robin to 8 cores
    (rank r -> core r%8, slot r//8) so every core sees an identical degree
    profile and one static NEFF schedule fits all cores. The 4 gather sources
    (scaled z / mid per layer) are exchanged with AllGather through shared
    DRAM as bf16 tables.
  - SpMV: ELL rounds per 128-node tile, PAIR-mode gather: table viewed as
    [V/2, 128] bf16 (two nodes per 256B row), idx = src>>1 fits a single int16
    window (25088 < 32768) -> no A/B window split, ~2.7% ELL padding only.
    Parity selection via a precomputed {0,1} bf16 mask multiply (DVE) with a
    stride-0 broadcast AP, then PE accumulates 8 sub-rounds per matmul into
    PSUM; gathers split across 4 SWDGE queues.
  - Padding gathers point at a zero pair (core0 pad slots 6250/6251).
"""
import os
import numpy as np
import ml_dtypes

BF16 = ml_dtypes.bfloat16
N, E, FIN, H, C = 50000, 800000, 100, 64, 40
NC = 8
P = 128
PC = 6272           # padded nodes per core (49*128; 6250 real)
T = PC // P         # 49 tiles
V = NC * PC         # 50176 table rows
VP = V // 2         # 25088 pair rows (int16-addressable)
PAD_PAIR = 3125     # pair (6250,6251): core0 zero pad rows
RB = 64             # rounds budget per gather batch


def _preprocess(x, edge_index):
    row = np.asarray(edge_index[0], dtype=np.int64)
    col = np.asarray(edge_index[1], dtype=np.int64)
    deg = np.bincount(row, minlength=N).astype(np.int64)
    order = np.argsort(-deg, kind="stable")        # rank -> node
    rank = np.empty(N, np.int64); rank[order] = np.arange(N)
    corer, posr = rank % NC, rank // NC
    znew = corer * PC + posr                       # node -> table row

    ecore = corer[row]; epos = posr[row]

    cnt = np.zeros((NC, PC), np.int64)
    np.add.at(cnt, (ecore, epos), 1)

    # shared static schedule: per-tile round counts = max over cores
    da = [int(cnt[:, t * P:(t + 1) * P].max()) for t in range(T)]

    # group tiles into gather batches with a round budget (SBUF staging cap)
    batches = []
    cur = []
    budget = 0
    for t in range(T):
        r = da[t]
        if cur and budget + r > RB:
            batches.append(cur); cur = []; budget = 0
        cur.append(t); budget += r
    if cur:
        batches.append(cur)

    ag = None
    ecz = znew[col]
    pad_pair = PAD_PAIR

    RMAX = int(cnt.max())
    ell = np.full((NC, PC, RMAX), pad_pair, np.int32)
    msk = np.zeros((NC, PC, RMAX, 2), np.float32)
    for c in range(NC):
        m = ecore == c
        ep, ez = epos[m], ecz[m]
        o = np.argsort(ep, kind="stable")
        ep, ez = ep[o], ez[o]
        starts = np.searchsorted(ep, np.arange(PC))
        r = np.arange(len(ep)) - starts[ep]
        ell[c, ep, r] = ez >> 1
        msk[c, ep, r, (ez & 1)] = 1.0

    # index buffer: idx k of a gather goes to [k%16, off + k//16]; replicated
    # to 128 partitions. k = r*128 + p -> stage[p, r].
    def wrap(seq):
        return np.tile(seq.reshape(-1, 16).T.astype(np.int16), (8, 1))

    ICOLS = 8 * sum(da)
    MC = sum(da)
    idxbuf = np.zeros((NC, 128, ICOLS), np.int16)
    mskbuf = np.zeros((NC, 128, 2 * MC), np.float32)
    offs = []      # per batch: (idx col, mask round offset)
    off = 0
    moff = 0
    for bt in batches:
        offs.append((off, moff))
        for c in range(NC):
            o = off
            mo = moff
            for t in bt:
                seq = ell[c, t * P:(t + 1) * P, :da[t]].T.ravel()
                idxbuf[c, :, o:o + 8 * da[t]] = wrap(seq)
                mskbuf[c, :, 2 * mo:2 * (mo + da[t])] = (
                    msk[c, t * P:(t + 1) * P, :da[t], :].reshape(P, 2 * da[t]))
                o += 8 * da[t]
                mo += da[t]
        off += 8 * sum(da[t] for t in bt)
        moff += sum(da[t] for t in bt)
    offs = (batches, offs)

    deg_pt = np.zeros((NC, P, T), np.float32)
    xts = np.zeros((NC, FIN + 1, PC), np.float32)
    for c in range(NC):
        nq = min(PC, (N - c + NC - 1) // NC)  # real nodes on this core (6250)
        nodes = order[np.arange(nq) * NC + c]
        q = np.arange(nq)
        deg_pt[c, q % P, q // P] = deg[nodes]
        xts[c, :FIN, :nq] = x[nodes].T
        xts[c, FIN, :nq] = 1.0
    return order, da, offs, ag, idxbuf, mskbuf.astype(BF16), deg_pt, xts


def _build(da, offs, ag):
    import concourse.bacc as bacc
    import concourse.mybir as mybir
    import concourse.tile as tile
    from concourse.masks import make_identity
    f32 = mybir.dt.float32
    bf16 = mybir.dt.bfloat16

    nc = bacc.Bacc("TRN2", target_bir_lowering=False, debug=False, num_devices=NC,
                   num_swdge_queues=4)
    IC = 8 * sum(da)
    MC = sum(da)
    xt_d = nc.dram_tensor("xt", [FIN + 1, PC], f32, kind="ExternalInput")
    w1_d = nc.dram_tensor("w1p", [FIN + 1, 3 * H], f32, kind="ExternalInput")
    w2_d = nc.dram_tensor("w2p", [H + 1, 3 * C], f32, kind="ExternalInput")
    idx_d = nc.dram_tensor("idx", [128, IC], mybir.dt.int16, kind="ExternalInput")
    msk_d = nc.dram_tensor("msk", [128, 2 * MC], bf16, kind="ExternalInput")
    deg_d = nc.dram_tensor("deg", [P, T], f32, kind="ExternalInput")
    out_d = nc.dram_tensor("out", [PC, C], f32, kind="ExternalOutput")

    with tile.TileContext(nc) as tc:
        with (
            tc.tile_pool(name="cst", bufs=1) as cst,
            tc.tile_pool(name="sb", bufs=4) as sb,
            tc.tile_pool(name="stg", bufs=4) as stg,
            tc.tile_pool(name="ps", bufs=2, space="PSUM") as ps,
            tc.tile_pool(name="acc", bufs=3, space="PSUM") as accp,
            tc.tile_pool(name="dram", bufs=1, space="DRAM") as dram,
        ):
            ident = cst.tile([P, P], f32)
            make_identity(nc, ident[:])
            identb = cst.tile([P, P], bf16)
            nc.vector.tensor_copy(out=identb[:], in_=ident[:])
            w1 = cst.tile([FIN + 1, 3 * H], f32)
            nc.sync.dma_start(out=w1[:], in_=w1_d[:])
            w2 = cst.tile([H + 1, 3 * C], f32)
            nc.sync.dma_start(out=w2[:], in_=w2_d[:])
            idx = cst.tile([128, IC], mybir.dt.int16)
            nc.sync.dma_start(out=idx[:], in_=idx_d[:])
            mskc = cst.tile([128, 2 * MC], bf16)
            nc.sync.dma_start(out=mskc[:], in_=msk_d[:])
            deg = cst.tile([P, T], f32)
            nc.sync.dma_start(out=deg[:], in_=deg_d[:])
            M = mybir.AluOpType.mult
            # dinv = (deg>0) / sqrt(max(deg,1))
            dinv = cst.tile([P, T], f32)
            dinv2 = cst.tile([P, T], f32)
            tmpd = cst.tile([P, T], f32)
            nc.vector.tensor_scalar(out=tmpd[:], in0=deg[:], scalar1=1.0, scalar2=None,
                                    op0=mybir.AluOpType.max)
            nc.vector.reciprocal(out=tmpd[:], in_=tmpd[:])
            nc.scalar.sqrt(out=tmpd[:], in_=tmpd[:])
            mk = cst.tile([P, T], f32)
            nc.vector.tensor_scalar(out=mk[:], in0=deg[:], scalar1=0.5, scalar2=None,
                                    op0=mybir.AluOpType.is_ge)
            nc.vector.tensor_mul(out=dinv[:], in0=tmpd[:], in1=mk[:])
            nc.vector.tensor_mul(out=dinv2[:], in0=dinv[:], in1=dinv[:])
            negd = cst.tile([P, T], f32)
            negd2 = cst.tile([P, T], f32)
            nc.vector.tensor_scalar(out=negd[:], in0=dinv[:], scalar1=-1.0,
                                    scalar2=None, op0=M)
            nc.vector.tensor_scalar(out=negd2[:], in0=dinv2[:], scalar1=-1.0,
                                    scalar2=None, op0=M)

            vt0buf = cst.tile([P, T, 2 * H], f32)   # [v | t0] per tile
            vt2buf = cst.tile([P, T, 2 * C], f32)   # [v2 | t02] per tile

            agin = [dram.tile([PC, H], bf16, tag=f"agin{i}", name=f"agin{i}")
                    for i in range(4)]
            tabs = [dram.tile([V, H], bf16, addr_space="Shared", tag=f"tab{i}",
                              name=f"tab{i}") for i in range(4)]

            def allgather(i):
                nc.gpsimd.collective_compute(
                    "AllGather", mybir.AluOpType.bypass,
                    replica_groups=[list(range(NC))],
                    ins=[agin[i][:].opt()], outs=[tabs[i][:].opt()])

            batches, boffs = offs

            def spmv(i, epilogue, agnext=None):
                """run spmv i over all tiles; epilogue(t, acc_psum) per tile;
                AllGather for table `agnext` fires after the last batch."""
                tabv = tabs[i][:].rearrange("(v two) c -> v (two c)", two=2)
                for g, bt in enumerate(batches):
                    rs = sum(da[t] for t in bt)
                    col, mo = boffs[g]
                    st = stg.tile([P, rs, 2 * H], bf16, tag="stage", name="st")
                    # split the batch's rounds across the 4 SWDGE queues
                    qsp = [rs * q // 4 for q in range(5)]
                    for q in range(4):
                        r0, r1 = qsp[q], qsp[q + 1]
                        if r1 == r0:
                            continue
                        nc.gpsimd.dma_gather(
                            out_ap=st[:, r0:r1, :], in_ap=tabv,
                            idxs_ap=idx[:, col + 8 * r0:col + 8 * r1],
                            num_idxs=(r1 - r0) * P, num_idxs_reg=(r1 - r0) * P,
                            elem_size=2 * H, single_packet=False, queue_num=q)
                    # parity select: st[p, r, s, c] *= msk[p, r, s]
                    stv4 = st[:].rearrange("p r (s c) -> p r s c", s=2)
                    mv4 = (mskc[:, 2 * mo:2 * (mo + rs)]
                           .rearrange("p (r s) -> p r s", s=2)
                           .unsqueeze(-1).broadcast_to([P, rs, 2, H]))
                    nc.vector.tensor_mul(out=stv4, in0=stv4, in1=mv4)
                    stv = st[:].rearrange("p r (s c) -> p (r s) c", s=2)
                    ao = 0
                    for t in bt:
                        sr = 2 * da[t]       # sub-rounds for this tile
                        p512 = accp.tile([P, 8, H], f32, tag="acc", space="PSUM",
                                         name="p512")
                        chunks = [(sc, min(8, sr - sc)) for sc in range(0, sr, 8)]
                        cov = chunks[0][1]
                        for k, (sc, r) in enumerate(chunks):
                            nc.tensor.matmul(out=p512[:, 0:r, :], lhsT=identb[:],
                                             rhs=stv[:, ao + sc:ao + sc + r, :],
                                             start=(k == 0),
                                             stop=(k == len(chunks) - 1))
                        acc = sb.tile([P, H], f32, tag="accs", name="accs")
                        nc.vector.tensor_reduce(
                            out=acc[:],
                            in_=p512[:, 0:cov, :].rearrange("p a b -> p b a"),
                            axis=mybir.AxisListType.X, op=mybir.AluOpType.add)
                        ao += sr
                        epilogue(t, acc)
                if agnext is not None:
                    allgather(agnext)

            dc = lambda t: dinv[:, t:t + 1]
            ndc = lambda t: negd[:, t:t + 1]
            nd2c = lambda t: negd2[:, t:t + 1]

            # ---- layer 1 matmuls: [2*W1[2] | W1[1] | W1[0]-W1[2] (+b1)] ----
            for t in range(T):
                xt = sb.tile([FIN + 1, P], f32, tag="xt")
                nc.sync.dma_start(out=xt[:], in_=xt_d[:, t * P:(t + 1) * P])
                pm = ps.tile([P, 3 * H], f32, tag="mm", space="PSUM")
                nc.tensor.matmul(out=pm[:], lhsT=xt[:], rhs=w1[:], start=True, stop=True)
                zt = sb.tile([P, H], bf16, tag="zt")
                nc.scalar.mul(out=zt[:], in_=pm[:, 0:H], mul=dc(t))
                nc.sync.dma_start(out=agin[0][t * P:(t + 1) * P, :], in_=zt[:])
                nc.vector.tensor_copy(out=vt0buf[:, t, :], in_=pm[:, H:3 * H])
            allgather(0)

            # ---- spmv1 -> mhat -> AG2 ----
            def epi1(t, acc):
                mh = sb.tile([P, H], f32, tag="mh", name="mh")
                nc.scalar.mul(out=mh[:], in_=acc[:], mul=nd2c(t))
                tv = sb.tile([P, H], f32, tag="tv", name="tv")
                nc.scalar.mul(out=tv[:], in_=vt0buf[:, t, 0:H], mul=dc(t))
                mhb = sb.tile([P, H], bf16, tag="mhb", name="mhb")
                nc.vector.tensor_add(out=mhb[:], in0=mh[:], in1=tv[:])
                nc.sync.dma_start(out=agin[1][t * P:(t + 1) * P, :], in_=mhb[:])
            spmv(0, epi1, agnext=1)

            # ---- spmv2 -> h = relu(t0 - dinv*s) -> layer 2 matmuls (fused) ----
            def epi2(t, acc):
                hb = sb.tile([P, H], f32, tag="hb", name="hb")
                nc.scalar.mul(out=hb[:], in_=acc[:], mul=ndc(t))
                nc.vector.tensor_add(out=hb[:], in0=hb[:], in1=vt0buf[:, t, H:2 * H])
                nc.vector.tensor_scalar(out=hb[:], in0=hb[:],
                                        scalar1=0.0, scalar2=None,
                                        op0=mybir.AluOpType.max)
                pt = ps.tile([H, P], f32, tag="tr", space="PSUM")
                nc.tensor.transpose(out=pt[:], in_=hb[:], identity=ident[:])
                ht = sb.tile([H + 1, P], f32, tag="ht")
                nc.vector.tensor_copy(out=ht[0:H, :], in_=pt[:])
                nc.vector.memset(ht[H:H + 1, :], 1.0)
                pm = ps.tile([P, 3 * C], f32, tag="mm", space="PSUM")
                nc.tensor.matmul(out=pm[:], lhsT=ht[:], rhs=w2[:], start=True, stop=True)
                z2 = sb.tile([P, H], bf16, tag="z2")
                nc.vector.memset(z2[:, C:H], 0.0)
                nc.scalar.mul(out=z2[:, 0:C], in_=pm[:, 0:C], mul=dc(t))
                nc.sync.dma_start(out=agin[2][t * P:(t + 1) * P, :], in_=z2[:])
                nc.vector.tensor_copy(out=vt2buf[:, t, :], in_=pm[:, C:3 * C])
            spmv(1, epi2, agnext=2)

            # ---- spmv3 -> mhat2 -> AG4 ----
            def epi3(t, acc):
                m2 = sb.tile([P, H], bf16, tag="m2", name="m2")
                nc.vector.memset(m2[:, C:H], 0.0)
                mt = sb.tile([P, C], f32, tag="mt", name="mt")
                nc.scalar.mul(out=mt[:], in_=acc[:, 0:C], mul=nd2c(t))
                tv = sb.tile([P, C], f32, tag="tv2", name="tv")
                nc.scalar.mul(out=tv[:], in_=vt2buf[:, t, 0:C], mul=dc(t))
                nc.vector.tensor_add(out=m2[:, 0:C], in0=mt[:], in1=tv[:])
                nc.sync.dma_start(out=agin[3][t * P:(t + 1) * P, :], in_=m2[:])
            spmv(2, epi3, agnext=3)

            # ---- spmv4 -> logits -> log_softmax -> out ----
            def epi4(t, acc):
                lg = sb.tile([P, C], f32, tag="lg", name="lg")
                nc.scalar.mul(out=lg[:], in_=acc[:, 0:C], mul=ndc(t))
                nc.vector.tensor_add(out=lg[:], in0=lg[:], in1=vt2buf[:, t, C:2 * C])
                nmx = sb.tile([P, 1], f32, tag="nmx", name="nmx")
                nc.vector.tensor_reduce(out=nmx[:], in_=lg[:],
                                        axis=mybir.AxisListType.X,
                                        op=mybir.AluOpType.max, negate=True)
                e1 = sb.tile([P, C], f32, tag="e1", name="e1")
                nc.scalar.activation(out=e1[:], in_=lg[:],
                                     func=mybir.ActivationFunctionType.Identity,
                                     bias=nmx[:], scale=1.0)
                ex = sb.tile([P, C], f32, tag="ex", name="ex")
                sm = sb.tile([P, 1], f32, tag="sm", name="sm")
                nc.scalar.activation(out=ex[:], in_=e1[:],
                                     func=mybir.ActivationFunctionType.Exp,
                                     accum_out=sm[:])
                rs = sb.tile([P, 1], f32, tag="rs", name="rs")
                nc.vector.reciprocal(out=rs[:], in_=sm[:])
                nls = sb.tile([P, 1], f32, tag="nls", name="nls")
                nc.scalar.activation(out=nls[:], in_=rs[:],
                                     func=mybir.ActivationFunctionType.Ln)
                ot = sb.tile([P, C], f32, tag="ot", name="ot")
                nc.scalar.activation(out=ot[:], in_=e1[:],
                                     func=mybir.ActivationFunctionType.Identity,
                                     bias=nls[:], scale=1.0)
                nc.sync.dma_start(out=out_d[t * P:(t + 1) * P, :], in_=ot[:])
            spmv(3, epi4)
    nc.compile()
    return nc


def kernel(x, edge_index, W1, b1, W2, b2):
    x = np.asarray(x, np.float32)
    W1 = np.asarray(W1, np.float32); b1 = np.asarray(b1, np.float32)
    W2 = np.asarray(W2, np.float32); b2 = np.asarray(b2, np.float32)

    order, da, offs, ag, idxbuf, mskbuf, deg_pt, xts = _preprocess(x, edge_index)

    w1p = np.zeros((FIN + 1, 3 * H), np.float32)
    w1p[:FIN, 0:H] = 2.0 * W1[2]
    w1p[:FIN, H:2 * H] = W1[1]
    w1p[:FIN, 2 * H:3 * H] = W1[0] - W1[2]
    w1p[FIN, 2 * H:3 * H] = b1
    w2p = np.zeros((H + 1, 3 * C), np.float32)
    w2p[:H, 0:C] = 2.0 * W2[2]
    w2p[:H, C:2 * C] = W2[1]
    w2p[:H, 2 * C:3 * C] = W2[0] - W2[2]
    w2p[H, 2 * C:3 * C] = b2

    trace = bool(os.environ.get("CHEB_TRACE"))
    if trace:
        import sys, types
        try:
            from trn_agent_boot.trn_boot import _ntff_profile_via_ctypes
            m = types.ModuleType("antenv.axon_hooks")
            m.get_axon_ntff_profile_hook = (
                lambda: _ntff_profile_via_ctypes("/opt/axon/libaxon_pjrt.so"))
            sys.modules["antenv.axon_hooks"] = m
        except Exception:
            trace = False

    nc = _build(da, offs, ag)
    from concourse.bass_utils import run_bass_kernel_spmd
    ins = [{"xt": xts[c], "w1p": w1p, "w2p": w2p, "idx": idxbuf[c],
            "msk": mskbuf[c], "deg": deg_pt[c]} for c in range(NC)]
    res = run_bass_kernel_spmd(nc, ins, core_ids=list(range(NC)), trace=trace)
    if trace and res.exec_time_ns is not None:
        print(f"HW exec time: {res.exec_time_ns} ns")

    out = np.empty((N, C), np.float32)
    for c in range(NC):
        nq = min(PC, (N - c + NC - 1) // NC)
        nodes = order[np.arange(nq) * NC + c]
        out[nodes] = res.results[c]["out"][:nq]
    return out
